# revision 1
# baseline (speedup 1.0000x reference)
"""Contrastive loss kernel for Trainium2 (8 NeuronCores, Bass/Tile).

Strategy (data-parallel over rows of embeddings1):
  - core c owns rows [c*CH, (c+1)*CH) of e1 ("i" index).
  - every core holds all of e2 (passed pre-transposed+bf16 from host) and
    computes the transposed logit tile  S_T[j, i] = <e2_j, e1n_i>  for all
    N j and its CH i's.  j lives on the partition axis, so the per-row
    scale 1/(T*||e2_j||) rides the ACT Exp `scale` vector, and the ACT
    `accum_out` gives the per-j partial column sums for free.
  - row sums (over all j) are partition-dim reductions done on the PE with
    a ones-vector stationary operand, accumulated in PSUM.
  - diagonal logits are computed separately as an exact f32 row-wise dot
    product e1n_i . e2_i (needs only the core's own CH rows of e2).
  - host combines: 8x partial colsums -> full column sums, subtracts the
    diagonal exp, takes logs and the two scalar sums.

Outputs per core: colp [128,JT] (column partial sums, j = jt*128+p),
rows [1,CH] (row sums incl. diagonal), ldiag [128,IT] (diag logits).
"""

import os
import sys

import numpy as np

for _p in ("/root/.axon_site", "/root/.axon_site/_ro/trn_rl_repo",
           "/root/.axon_site/_ro/pypackages", "/opt/trn_rl_repo"):
    if os.path.isdir(_p) and _p not in sys.path:
        sys.path.append(_p)

import ml_dtypes

N, D = 4096, 1024
NCORES = 8
CH = N // NCORES          # 512 rows of e1 per core
INV_T = 10.0              # 1 / temperature

_CACHE = {}


def _legalize_waits(nc, cap=1):
    """Split >cap semaphore waits per instruction onto preceding NOPs.

    The walrus build in this container rejects instructions carrying more
    than ~2 sync waits ("Too many sync wait commands"); Tile emits up to
    12 on the final barrier drain.  Hoisting the excess waits onto NOPs
    issued just before, on the same engine queue, is semantics-preserving
    (the engine is in-order, so waiting earlier is safe).
    """
    import concourse.mybir as mybir
    nid = 0
    for f in nc.m.functions:
        for b in f.blocks:
            insts = b.instructions
            i = 0
            while i < len(insts):
                inst = insts[i]
                si = inst.sync_info
                if si is not None and si.on_wait and len(si.on_wait) > cap:
                    waits = list(si.on_wait)
                    inst.sync_info = mybir.SyncInfo(
                        on_wait=waits[-cap:], on_update=list(si.on_update))
                    excess = waits[:-cap]
                    pos = i
                    for j in range(0, len(excess), cap):
                        nop = mybir.InstNoOp(
                            name=f"I-waitnop-{nid}", ins=[], outs=[])
                        nid += 1
                        nop.engine = inst.engine
                        nop.sync_info = mybir.SyncInfo(
                            on_wait=excess[j:j + cap], on_update=[])
                        insts.insert(pos, nop)
                        pos += 1
                        i += 1
                i += 1
    return nc


def build_nc(n=N, d=D, ch=CH, legalize=True):
    import concourse.bass as bass
    import concourse.mybir as mybir
    import concourse.tile as tile
    from concourse.masks import make_identity
    from contextlib import ExitStack

    fp32 = mybir.dt.float32
    bf16 = mybir.dt.bfloat16
    AF = mybir.ActivationFunctionType
    OP = mybir.AluOpType
    ts = bass.ts

    kt = d // 128             # contraction tiles
    jt_n = n // 128           # j tiles
    it_n = ch // 128          # i tiles

    nc = bass.Bass(trn_type="TRN2")
    e1c_d = nc.dram_tensor("e1c", [ch, d], fp32, kind="ExternalInput")
    e2c_d = nc.dram_tensor("e2c", [ch, d], fp32, kind="ExternalInput")
    e2t_d = nc.dram_tensor("e2t", [d, n], bf16, kind="ExternalInput")
    e2r_d = nc.dram_tensor("e2r", [n, d], bf16, kind="ExternalInput")
    colp_d = nc.dram_tensor("colp", [128, jt_n], fp32, kind="ExternalOutput")
    rows_d = nc.dram_tensor("rows", [1, ch], fp32, kind="ExternalOutput")
    ldiag_d = nc.dram_tensor("ldiag", [128, it_n], fp32, kind="ExternalOutput")

    with ExitStack() as ctx:
        tc = ctx.enter_context(tile.TileContext(nc))
        res = ctx.enter_context(tc.tile_pool(name="res", bufs=1))
        loadp = ctx.enter_context(tc.tile_pool(name="loadp", bufs=3))
        junkp = ctx.enter_context(tc.tile_pool(name="junkp", bufs=3))
        smallp = ctx.enter_context(tc.tile_pool(name="smallp", bufs=4))
        e1np = ctx.enter_context(tc.tile_pool(name="e1np", bufs=2))
        pml = ctx.enter_context(tc.tile_pool(name="pml", bufs=4, space="PSUM"))
        ptr = ctx.enter_context(tc.tile_pool(name="ptr", bufs=2, space="PSUM"))
        prow = ctx.enter_context(tc.tile_pool(name="prow", bufs=1, space="PSUM"))

        # resident SBUF tensors
        e2t_sb = res.tile([128, kt, n], bf16)     # e2^T, d on partitions
        e1t_sb = res.tile([128, kt, ch], bf16)    # normalized e1^T
        exps_sb = res.tile([128, jt_n, ch], bf16)  # exp(logits^T)
        e1f_all = res.tile([128, it_n, d], fp32)  # raw e1 rows (f32)
        colp_sb = res.tile([128, jt_n], fp32)
        ss2_sb = res.tile([128, jt_n], fp32)      # sumsq of all e2 rows
        srow = res.tile([128, jt_n], fp32)        # 10 / ||e2_j||
        norm2 = res.tile([128, jt_n], fp32)
        ldiag_sb = res.tile([128, it_n], fp32)
        ss1 = res.tile([128, it_n], fp32)
        ssc = res.tile([128, it_n], fp32)
        raw = res.tile([128, it_n], fp32)
        r1 = res.tile([128, it_n], fp32)
        rc = res.tile([128, it_n], fp32)
        rows_sb = res.tile([1, ch], fp32)
        ident = res.tile([128, 128], bf16)
        ones_bf = res.tile([128, 1], bf16)

        make_identity(nc, ident)
        nc.vector.memset(ones_bf, 1.0)

        # ---- load e2^T (stationary operand of the big matmul) ----
        for k in range(kt):
            nc.sync.dma_start(out=e2t_sb[:, k, :], in_=e2t_d[ts(k, 128), :])

        # ---- e1 rows: sumsq, diag dot with e2 rows ----
        for t in range(it_n):
            nc.sync.dma_start(out=e1f_all[:, t, :], in_=e1c_d[ts(t, 128), :])
        for t in range(it_n):
            e2f = loadp.tile([128, d], fp32, tag="e2f")
            nc.sync.dma_start(out=e2f, in_=e2c_d[ts(t, 128), :])
            junkc = junkp.tile([128, d], fp32, tag="junkc")
            nc.scalar.activation(out=junkc, in_=e1f_all[:, t, :],
                                 func=AF.Square, accum_out=ss1[:, t:t + 1])
            junkd = junkp.tile([128, d], fp32, tag="junkc")
            nc.scalar.activation(out=junkd, in_=e2f, func=AF.Square,
                                 accum_out=ssc[:, t:t + 1])
            junke = junkp.tile([128, d], fp32, tag="junkc")
            nc.vector.tensor_mul(out=junke, in0=e1f_all[:, t, :], in1=e2f)
            nc.vector.reduce_sum(out=raw[:, t:t + 1], in_=junke,
                                 axis=mybir.AxisListType.X)

        def rsqrt_nr(dst, ss):
            # dst = 1/sqrt(ss), Newton-refined to fp32 accuracy
            a = smallp.tile([128, it_n], fp32, tag="nr_a")
            nc.scalar.activation(out=a, in_=ss, func=AF.Ln)
            nc.scalar.activation(out=dst, in_=a, func=AF.Exp, scale=-0.5)
            b = smallp.tile([128, it_n], fp32, tag="nr_b")
            nc.vector.tensor_mul(out=b, in0=dst, in1=dst)
            nc.vector.tensor_mul(out=b, in0=b, in1=ss)
            nc.vector.tensor_scalar(out=b, in0=b, scalar1=-0.5, scalar2=1.5,
                                    op0=OP.mult, op1=OP.add)
            nc.vector.tensor_mul(out=dst, in0=dst, in1=b)

        rsqrt_nr(r1, ss1)
        rsqrt_nr(rc, ssc)
        # ldiag = raw * r1 * rc * 10
        m = smallp.tile([128, it_n], fp32, tag="nr_m")
        nc.vector.tensor_mul(out=m, in0=r1, in1=rc)
        nc.vector.tensor_mul(out=ldiag_sb, in0=raw, in1=m)
        nc.vector.tensor_scalar_mul(out=ldiag_sb, in0=ldiag_sb, scalar1=INV_T)
        nc.sync.dma_start(out=ldiag_d[:, :], in_=ldiag_sb)

        # ---- normalized e1 -> bf16 -> transpose onto e1t_sb ----
        for t in range(it_n):
            e1n = e1np.tile([128, d], bf16, tag="e1n")
            nc.vector.tensor_scalar_mul(out=e1n, in0=e1f_all[:, t, :],
                                        scalar1=r1[:, t:t + 1])
            for k in range(kt):
                ptile = ptr.tile([128, 128], bf16, tag="ptile")
                nc.tensor.transpose(out=ptile, in_=e1n[:, ts(k, 128)],
                                    identity=ident)
                nc.vector.tensor_copy(out=e1t_sb[:, k, ts(t, 128)], in_=ptile)

        # ---- sumsq of every e2 row ----
        # alternate engines: ACT Square(+accum) / GPSIMD square + DVE reduce
        for jt in range(jt_n):
            e2rt = loadp.tile([128, d], bf16, tag="e2rt")
            nc.sync.dma_start(out=e2rt, in_=e2r_d[ts(jt, 128), :])
            if jt % 2 == 0:
                junka = junkp.tile([128, d], bf16, tag="junka")
                nc.scalar.activation(out=junka, in_=e2rt, func=AF.Square,
                                     accum_out=ss2_sb[:, jt:jt + 1])
            else:
                junkb = junkp.tile([128, d], bf16, tag="junkb")
                nc.gpsimd.tensor_mul(out=junkb, in0=e2rt, in1=e2rt)
                nc.vector.reduce_sum(out=ss2_sb[:, jt:jt + 1], in_=junkb,
                                     axis=mybir.AxisListType.X)
        # srow = 10 / sqrt(ss2), in groups of 8 j-tiles to unblock the pipe
        g = 8 if jt_n % 8 == 0 else jt_n
        for j0 in range(0, jt_n, g):
            sl = slice(j0, j0 + g)
            nc.scalar.activation(out=norm2[:, sl], in_=ss2_sb[:, sl],
                                 func=AF.Ln)
            nc.scalar.activation(out=srow[:, sl], in_=norm2[:, sl],
                                 func=AF.Exp, scale=-0.5)
            nc.vector.tensor_scalar_mul(out=srow[:, sl], in0=srow[:, sl],
                                        scalar1=INV_T)

        # ---- main loop: 128-row j blocks of the transposed logit tile ----
        for jt in range(jt_n):
            pl = pml.tile([128, ch], fp32, tag="pl")
            for k in range(kt):
                nc.tensor.matmul(pl, lhsT=e2t_sb[:, k, ts(jt, 128)],
                                 rhs=e1t_sb[:, k, :],
                                 start=(k == 0), stop=(k == kt - 1))
            nc.scalar.activation(out=exps_sb[:, jt, :], in_=pl, func=AF.Exp,
                                 scale=srow[:, jt:jt + 1],
                                 accum_out=colp_sb[:, jt:jt + 1])

        # ---- row sums: ones^T @ exps, accumulated over all j tiles ----
        prow_t = prow.tile([1, ch], fp32)
        for jt in range(jt_n):
            nc.tensor.matmul(prow_t, lhsT=ones_bf, rhs=exps_sb[:, jt, :],
                             start=(jt == 0), stop=(jt == jt_n - 1))
        nc.scalar.copy(out=rows_sb, in_=prow_t)

        nc.sync.dma_start(out=rows_d[:, :], in_=rows_sb)
        nc.sync.dma_start(out=colp_d[:, :], in_=colp_sb)
    return _legalize_waits(nc) if legalize else nc


def _get_nc():
    if "nc" not in _CACHE:
        _CACHE["nc"] = build_nc()
    return _CACHE["nc"]


def _run(in_maps, trace=False, **kw):
    from concourse.bass_utils import run_bass_kernel_spmd
    return run_bass_kernel_spmd(_get_nc(), in_maps,
                                core_ids=list(range(NCORES)),
                                trace=trace, **kw)


def kernel(embeddings1, embeddings2, _trace=False, _full_result=False):
    e1 = np.ascontiguousarray(np.asarray(embeddings1, dtype=np.float32))
    e2 = np.ascontiguousarray(np.asarray(embeddings2, dtype=np.float32))
    assert e1.shape == (N, D) and e2.shape == (N, D)
    bf = ml_dtypes.bfloat16
    e2_bf = e2.astype(bf)
    e2t = np.ascontiguousarray(e2_bf.T)

    in_maps = []
    for c in range(NCORES):
        sl = slice(c * CH, (c + 1) * CH)
        in_maps.append({
            "e1c": np.ascontiguousarray(e1[sl]),
            "e2c": np.ascontiguousarray(e2[sl]),
            "e2t": e2t,
            "e2r": e2_bf,
        })
    bres = _run(in_maps, trace=_trace)
    outs = bres.results

    ldiag = np.concatenate(
        [np.asarray(o["ldiag"], dtype=np.float64).T.reshape(-1) for o in outs])
    rows = np.concatenate(
        [np.asarray(o["rows"], dtype=np.float64).reshape(-1) for o in outs])
    colsum = np.zeros(N, dtype=np.float64)
    for o in outs:
        colsum += np.asarray(o["colp"], dtype=np.float64).T.reshape(-1)

    ed = np.exp(ldiag)
    row_denom = rows - ed
    col_denom = colsum - ed
    sim12 = float(np.sum(ldiag - np.log(row_denom)))
    sim21 = float(np.sum(ldiag - np.log(col_denom)))
    result = (np.float32(-sim12), np.float32(-sim21))
    if _full_result:
        return result, bres
    return result



# revision 5
# speedup vs baseline: 2.0539x; 2.0539x over previous
"""Contrastive loss kernel for Trainium2 (8 NeuronCores, Bass/Tile).

v2 strategy (data-parallel over rows of embeddings1, fp8 DoubleRow matmul):
  - Host normalizes both embedding sets (f64), computes the diagonal logits
    exactly, scales by 16 and quantizes to fp8e4m3.  Each core gets its own
    512 normalized e1 rows (transposed, k-slab layout) plus all of e2
    (transposed, k-slab layout).
  - Core c computes its [512, 4096] logit tile  S[i, j] = 256*<e1n_i, e2n_j>
    with fp8 DoubleRow matmuls (contraction 2x128 per step, 4 steps for
    D=1024), accumulating [128, 512] PSUM chunks.
  - ACT applies exp(0.0390625 * psum) (= exp(10 * cos sim)), writing bf16
    exps to SBUF; the free ACT accum gives per-i partial row sums.
  - Column partial sums (over the core's 512 i) come from ones^T @ exps
    matmuls accumulated in PSUM, DMA'd straight from PSUM to DRAM.
  - Host combines: row sums, 8x column partials, exact diag -> two scalars.

Outputs per core: rows [128, 32] (accum partials, i = ib*128+p, col ib*8+jc),
colp [8, 512] (column partials, j = jc*512 + col).
"""

import os
import sys

import numpy as np

for _p in ("/root/.axon_site", "/root/.axon_site/_ro/trn_rl_repo",
           "/root/.axon_site/_ro/pypackages", "/opt/trn_rl_repo"):
    if os.path.isdir(_p) and _p not in sys.path:
        sys.path.append(_p)

import ml_dtypes

N, D = 4096, 1024
NCORES = 8
CH = N // NCORES          # 512 rows of e1 per core
KT = D // 128             # 8 contraction subtiles
IBT = CH // 128           # 4 i-blocks per core
JCW = 512                 # j chunk width (one PSUM bank)
JCT = N // JCW            # 8 j chunks
SC = 16.0                 # fp8 pre-scale; logits = psum * 10/SC^2
ACT_SCALE = 10.0 / (SC * SC)

_CACHE = {}


def _legalize_waits(nc, cap=1):
    """Split >cap semaphore waits per instruction onto preceding NOPs."""
    import concourse.mybir as mybir
    nid = 0
    for f in nc.m.functions:
        for b in f.blocks:
            insts = b.instructions
            i = 0
            while i < len(insts):
                inst = insts[i]
                si = inst.sync_info
                if si is not None and si.on_wait and len(si.on_wait) > cap:
                    waits = list(si.on_wait)
                    inst.sync_info = mybir.SyncInfo(
                        on_wait=waits[-cap:], on_update=list(si.on_update))
                    excess = waits[:-cap]
                    pos = i
                    for j in range(0, len(excess), cap):
                        nop = mybir.InstNoOp(
                            name=f"I-waitnop-{nid}", ins=[], outs=[])
                        nid += 1
                        nop.engine = inst.engine
                        nop.sync_info = mybir.SyncInfo(
                            on_wait=excess[j:j + cap], on_update=[])
                        insts.insert(pos, nop)
                        pos += 1
                        i += 1
                i += 1
    return nc


def build_nc(legalize=True):
    import concourse.bass as bass
    import concourse.mybir as mybir
    import concourse.tile as tile
    from contextlib import ExitStack

    fp32 = mybir.dt.float32
    bf16 = mybir.dt.bfloat16
    fp8 = mybir.dt.float8e4
    AF = mybir.ActivationFunctionType
    DR = mybir.MatmulPerfMode.DoubleRow

    nc = bass.Bass(trn_type="TRN2")
    e1t_d = nc.dram_tensor("e1t", [KT, 128, CH], fp8, kind="ExternalInput")
    e2t_d = nc.dram_tensor("e2t", [KT, 128, N], fp8, kind="ExternalInput")
    rows_d = nc.dram_tensor("rows", [128, IBT * JCT], fp32,
                            kind="ExternalOutput")
    colp_d = nc.dram_tensor("colp", [JCT, JCW], fp32, kind="ExternalOutput")

    with ExitStack() as ctx:
        tc = ctx.enter_context(tile.TileContext(nc))
        res = ctx.enter_context(tc.tile_pool(name="res", bufs=1))
        pmm = ctx.enter_context(tc.tile_pool(name="pmm", bufs=4, space="PSUM"))
        pcol = ctx.enter_context(tc.tile_pool(name="pcol", bufs=1,
                                              space="PSUM"))
        pjunk = ctx.enter_context(tc.tile_pool(name="pjunk", bufs=1,
                                               space="PSUM"))

        e2t_sb = res.tile([128, KT, N], fp8)        # 32 KiB/part
        e1t_sb = res.tile([128, KT, CH], fp8)       # 4 KiB/part
        exps_sb = res.tile([128, IBT, N], bf16)     # 32 KiB/part
        rows_sb = res.tile([128, IBT * JCT], fp32)
        colp_sb = res.tile([JCT, JCW], fp32)
        # mask8[:, jc, :] = [128, 8] with column jc all-ones: routes the
        # ones-matmul partial for j-chunk jc onto PSUM partition jc.
        mask8 = res.tile([128, JCT, JCT], bf16)
        nc.vector.memset(mask8, 0.0)
        for jc in range(JCT):
            nc.vector.memset(mask8[:, jc, jc:jc + 1], 1.0)

        # ---- input DMAs, k-slab order ----
        for k in range(KT):
            nc.sync.dma_start(out=e1t_sb[:, k, :], in_=e1t_d[k])
        for k in range(KT):
            nc.sync.dma_start(out=e2t_sb[:, k, :], in_=e2t_d[k])

        # ---- HAM warmup: junk DR matmuls on e1t while e2t streams in ----
        for w in range(16):
            pj = pjunk.tile([128, JCW], fp32, tag="pj")
            nc.tensor.matmul(pj, lhsT=e1t_sb[:, 0:2, 0:128],
                             rhs=e1t_sb[:, 0:2, 0:512],
                             start=True, stop=True, perf_mode=DR)

        # ---- main: S[i, j] tiles, exp, row-sum accum ----
        for ib in range(IBT):
            isl = slice(ib * 128, (ib + 1) * 128)
            for jc in range(JCT):
                jsl = slice(jc * JCW, (jc + 1) * JCW)
                pl = pmm.tile([128, JCW], fp32, tag="pl")
                for q in range(KT // 2):
                    nc.tensor.matmul(pl,
                                     lhsT=e1t_sb[:, 2 * q:2 * q + 2, isl],
                                     rhs=e2t_sb[:, 2 * q:2 * q + 2, jsl],
                                     start=(q == 0), stop=(q == KT // 2 - 1),
                                     perf_mode=DR)
                idx = ib * JCT + jc
                nc.scalar.activation(out=exps_sb[:, ib, jsl], in_=pl,
                                     func=AF.Exp, scale=ACT_SCALE,
                                     accum_out=rows_sb[:, idx:idx + 1])

        # ---- column partials: masked-ones^T @ exps, one [8, 512] PSUM ----
        pc = pcol.tile([JCT, JCW], fp32, tag="pc")
        nmm = IBT * JCT
        i = 0
        for ib in range(IBT):
            for jc in range(JCT):
                jsl = slice(jc * JCW, (jc + 1) * JCW)
                nc.tensor.matmul(pc, lhsT=mask8[:, jc, :],
                                 rhs=exps_sb[:, ib, jsl],
                                 start=(i == 0), stop=(i == nmm - 1))
                i += 1
        nc.vector.tensor_copy(out=colp_sb, in_=pc)
        nc.sync.dma_start(out=colp_d[:, :], in_=colp_sb)

        nc.sync.dma_start(out=rows_d[:, :], in_=rows_sb)
    return _legalize_waits(nc) if legalize else nc


def _get_nc():
    if "nc" not in _CACHE:
        _CACHE["nc"] = build_nc()
    return _CACHE["nc"]


def _run(in_maps, trace=False, **kw):
    from concourse.bass_utils import run_bass_kernel_spmd
    return run_bass_kernel_spmd(_get_nc(), in_maps,
                                core_ids=list(range(NCORES)),
                                trace=trace, **kw)


def _prep(embeddings1, embeddings2):
    e1 = np.asarray(embeddings1, dtype=np.float64)
    e2 = np.asarray(embeddings2, dtype=np.float64)
    e1n = e1 / np.maximum(np.linalg.norm(e1, axis=1, keepdims=True), 1e-12)
    e2n = e2 / np.maximum(np.linalg.norm(e2, axis=1, keepdims=True), 1e-12)
    ldiag = 10.0 * np.einsum("nd,nd->n", e1n, e2n)
    fp8 = ml_dtypes.float8_e4m3
    q1 = (e1n * SC).astype(np.float32).astype(fp8)
    q2 = (e2n * SC).astype(np.float32).astype(fp8)
    # k-slab layouts: [KT, 128, cols], slab k holds contraction rows k*128..
    e2t = np.ascontiguousarray(q2.T.reshape(KT, 128, N))
    e1ts = []
    for c in range(NCORES):
        sl = slice(c * CH, (c + 1) * CH)
        e1ts.append(np.ascontiguousarray(q1[sl].T.reshape(KT, 128, CH)))
    return e1ts, e2t, ldiag


def kernel(embeddings1, embeddings2, _trace=False, _full_result=False):
    e1ts, e2t, ldiag = _prep(embeddings1, embeddings2)
    in_maps = [{"e1t": e1ts[c], "e2t": e2t} for c in range(NCORES)]
    bres = _run(in_maps, trace=_trace)
    outs = bres.results

    rows = np.empty(N, dtype=np.float64)
    colsum = np.zeros(N, dtype=np.float64)
    for c, o in enumerate(outs):
        r = np.asarray(o["rows"], dtype=np.float64)  # [128, IBT*JCT]
        for ib in range(IBT):
            part = r[:, ib * JCT:(ib + 1) * JCT].sum(axis=1)
            rows[c * CH + ib * 128:c * CH + (ib + 1) * 128] = part
        colsum += np.asarray(o["colp"], dtype=np.float64).reshape(-1)

    ed = np.exp(ldiag)
    row_denom = rows - ed
    col_denom = colsum - ed
    sim12 = float(np.sum(ldiag - np.log(row_denom)))
    sim21 = float(np.sum(ldiag - np.log(col_denom)))
    result = (np.float32(-sim12), np.float32(-sim21))
    if _full_result:
        return result, bres
    return result


# revision 7
# speedup vs baseline: 2.6311x; 1.2810x over previous
"""Contrastive loss kernel for Trainium2 (8 NeuronCores, Bass/Tile).

v3 strategy (data-parallel over rows of embeddings1, fp8 DoubleRow matmul):
  - Host normalizes both embedding sets (f64), computes the diagonal logits
    exactly, scales by 16 and quantizes to fp8e4m3.  Each core gets its own
    512 normalized e1 rows (transposed, k-slab layout) plus all of e2
    (transposed, k-slab layout).
  - Core c computes its [512, 4096] logit tile  S[i, j] = 256*<e1n_i, e2n_j>
    with fp8 DoubleRow matmuls (contraction 2x128 per step, 4 steps for
    D=1024).  Loop order (ib, jc-group, q, jc) keeps one stationary operand
    across 4 moving matmuls so LDWEIGHTS stays hidden.
  - ACT applies exp(0.0390625 * psum) (= exp(10 * cos sim)), writing fp8
    exps to SBUF; the free ACT accum gives per-i partial row sums.
  - Column partials (over the core's 512 i) come from 16 DoubleRow matmuls
    with one-hot mask weights routing j-chunk jc onto PSUM partition jc of
    a single [8, 512] accumulator.
  - HAM warmup: 8 junk matmuls on a memset tile (no DMA dependency) flip
    the PE clock gate to 8/8 while e2 streams in.

Outputs per core: rows [128, 32] (accum partials, i = ib*128+p, col ib*8+jc),
colp [8, 512] (column partials, j = jc*512 + col).
"""

import os
import sys

import numpy as np

for _p in ("/root/.axon_site", "/root/.axon_site/_ro/trn_rl_repo",
           "/root/.axon_site/_ro/pypackages", "/opt/trn_rl_repo"):
    if os.path.isdir(_p) and _p not in sys.path:
        sys.path.append(_p)

import ml_dtypes

N, D = 4096, 1024
NCORES = 8
CH = N // NCORES          # 512 rows of e1 per core
KT = D // 128             # 8 contraction subtiles
IBT = CH // 128           # 4 i-blocks per core
JCW = 512                 # j chunk width (one PSUM bank)
JCT = N // JCW            # 8 j chunks
SC = 16.0                 # fp8 pre-scale; logits = psum * 10/SC^2
ACT_SCALE = 10.0 / (SC * SC)

_CACHE = {}


def _legalize_waits(nc, cap=1):
    """Split >cap semaphore waits per instruction onto preceding NOPs."""
    import concourse.mybir as mybir
    nid = 0
    for f in nc.m.functions:
        for b in f.blocks:
            insts = b.instructions
            i = 0
            while i < len(insts):
                inst = insts[i]
                si = inst.sync_info
                if si is not None and si.on_wait and len(si.on_wait) > cap:
                    waits = list(si.on_wait)
                    inst.sync_info = mybir.SyncInfo(
                        on_wait=waits[-cap:], on_update=list(si.on_update))
                    excess = waits[:-cap]
                    pos = i
                    for j in range(0, len(excess), cap):
                        nop = mybir.InstNoOp(
                            name=f"I-waitnop-{nid}", ins=[], outs=[])
                        nid += 1
                        nop.engine = inst.engine
                        nop.sync_info = mybir.SyncInfo(
                            on_wait=excess[j:j + cap], on_update=[])
                        insts.insert(pos, nop)
                        pos += 1
                        i += 1
                i += 1
    return nc


def build_nc(legalize=True):
    import concourse.bass as bass
    import concourse.mybir as mybir
    import concourse.tile as tile
    from contextlib import ExitStack

    fp32 = mybir.dt.float32
    fp8 = mybir.dt.float8e4
    AF = mybir.ActivationFunctionType
    DR = mybir.MatmulPerfMode.DoubleRow

    nc = bass.Bass(trn_type="TRN2")
    e1t_d = nc.dram_tensor("e1t", [128, KT * CH], fp8, kind="ExternalInput")
    e2t_d = nc.dram_tensor("e2t", [KT, 128, N], fp8, kind="ExternalInput")
    rows_d = nc.dram_tensor("rows", [128, IBT * JCT], fp32,
                            kind="ExternalOutput")
    colp_d = nc.dram_tensor("colp", [JCT, JCW], fp32, kind="ExternalOutput")

    with ExitStack() as ctx:
        tc = ctx.enter_context(tile.TileContext(nc))
        res = ctx.enter_context(tc.tile_pool(name="res", bufs=1))
        pmm = ctx.enter_context(tc.tile_pool(name="pmm", bufs=7, space="PSUM"))
        pcol = ctx.enter_context(tc.tile_pool(name="pcol", bufs=1,
                                              space="PSUM"))

        e2t_sb = res.tile([128, KT, N], fp8)        # 32 KiB/part
        e1t_sb = res.tile([128, KT, CH], fp8)       # 4 KiB/part
        exps_sb = res.tile([128, IBT, N], fp8)      # 16 KiB/part
        rows_sb = res.tile([128, IBT * JCT], fp32)
        colp_sb = res.tile([JCT, JCW], fp32)
        jnk = res.tile([128, 2, JCW], fp8)          # warmup operand
        # mask4d[:, :, jc, :] = [128, 2, 8] DR pair with column jc all-ones:
        # routes the ones-matmul partial for j-chunk jc onto PSUM partition jc.
        mask4d = res.tile([128, 2, JCT, JCT], fp8)
        nc.vector.memset(jnk, 0.0)
        nc.vector.memset(mask4d, 0.0)
        for jc in range(JCT):
            nc.vector.memset(mask4d[:, 0, jc, jc:jc + 1], 1.0)
            nc.vector.memset(mask4d[:, 1, jc, jc:jc + 1], 1.0)

        # ---- HAM warmup: junk DR matmuls, no DMA dependency ----
        for w in range(8):
            pj = pmm.tile([128, JCW], fp32, tag="pl")
            nc.tensor.matmul(pj, lhsT=jnk[:, :, 0:128], rhs=jnk[:, :, :],
                             start=True, stop=True, perf_mode=DR)

        # ---- input DMAs, k-slab order ----
        nc.sync.dma_start(out=e1t_sb[:, :, :], in_=e1t_d[:, :])
        for k in range(KT):
            nc.sync.dma_start(out=e2t_sb[:, k, :], in_=e2t_d[k])

        # ---- main: S[i, j] tiles, exp, row-sum accum ----
        JG = 4                      # j-chunks per stationary group
        for ib in range(IBT):
            isl = slice(ib * 128, (ib + 1) * 128)
            for jg in range(JCT // JG):
                pls = []
                for q in range(KT // 2):
                    lhsT = e1t_sb[:, 2 * q:2 * q + 2, isl]
                    for j4 in range(JG):
                        jc = jg * JG + j4
                        jsl = slice(jc * JCW, (jc + 1) * JCW)
                        if q == 0:
                            pls.append(pmm.tile([128, JCW], fp32, tag="pl",
                                                name=f"pl_{ib}_{jg}_{j4}"))
                        nc.tensor.matmul(pls[j4], lhsT=lhsT,
                                         rhs=e2t_sb[:, 2 * q:2 * q + 2, jsl],
                                         start=(q == 0),
                                         stop=(q == KT // 2 - 1),
                                         perf_mode=DR)
                for j4 in range(JG):
                    jc = jg * JG + j4
                    jsl = slice(jc * JCW, (jc + 1) * JCW)
                    idx = ib * JCT + jc
                    nc.scalar.activation(out=exps_sb[:, ib, jsl], in_=pls[j4],
                                         func=AF.Exp, scale=ACT_SCALE,
                                         accum_out=rows_sb[:, idx:idx + 1])

        nc.sync.dma_start(out=rows_d[:, :], in_=rows_sb)

        # ---- column partials: masked-ones^T @ exps, one [8, 512] PSUM ----
        pc = pcol.tile([JCT, JCW], fp32, tag="pc")
        nmm = (IBT // 2) * JCT
        i = 0
        for a in range(IBT // 2):
            for jc in range(JCT):
                jsl = slice(jc * JCW, (jc + 1) * JCW)
                nc.tensor.matmul(pc, lhsT=mask4d[:, :, jc, :],
                                 rhs=exps_sb[:, 2 * a:2 * a + 2, jsl],
                                 start=(i == 0), stop=(i == nmm - 1),
                                 perf_mode=DR)
                i += 1
        nc.vector.tensor_copy(out=colp_sb, in_=pc)
        nc.sync.dma_start(out=colp_d[:, :], in_=colp_sb)
    return _legalize_waits(nc) if legalize else nc


def _get_nc():
    if "nc" not in _CACHE:
        _CACHE["nc"] = build_nc()
    return _CACHE["nc"]


def _run(in_maps, trace=False, **kw):
    from concourse.bass_utils import run_bass_kernel_spmd
    return run_bass_kernel_spmd(_get_nc(), in_maps,
                                core_ids=list(range(NCORES)),
                                trace=trace, **kw)


def _prep(embeddings1, embeddings2):
    e1 = np.asarray(embeddings1, dtype=np.float64)
    e2 = np.asarray(embeddings2, dtype=np.float64)
    e1n = e1 / np.maximum(np.linalg.norm(e1, axis=1, keepdims=True), 1e-12)
    e2n = e2 / np.maximum(np.linalg.norm(e2, axis=1, keepdims=True), 1e-12)
    ldiag = 10.0 * np.einsum("nd,nd->n", e1n, e2n)
    fp8 = ml_dtypes.float8_e4m3
    q1 = (e1n * SC).astype(np.float32).astype(fp8)
    q2 = (e2n * SC).astype(np.float32).astype(fp8)
    # e2: k-slab layout [KT, 128, N]; e1: SBUF layout [128, KT*CH]
    e2t = np.ascontiguousarray(q2.T.reshape(KT, 128, N))
    e1ts = []
    for c in range(NCORES):
        sl = slice(c * CH, (c + 1) * CH)
        p = q1[sl].T.reshape(KT, 128, CH).transpose(1, 0, 2)
        e1ts.append(np.ascontiguousarray(p.reshape(128, KT * CH)))
    return e1ts, e2t, ldiag


def kernel(embeddings1, embeddings2, _trace=False, _full_result=False):
    e1ts, e2t, ldiag = _prep(embeddings1, embeddings2)
    in_maps = [{"e1t": e1ts[c], "e2t": e2t} for c in range(NCORES)]
    bres = _run(in_maps, trace=_trace)
    outs = bres.results

    rows = np.empty(N, dtype=np.float64)
    colsum = np.zeros(N, dtype=np.float64)
    for c, o in enumerate(outs):
        r = np.asarray(o["rows"], dtype=np.float64)  # [128, IBT*JCT]
        for ib in range(IBT):
            part = r[:, ib * JCT:(ib + 1) * JCT].sum(axis=1)
            rows[c * CH + ib * 128:c * CH + (ib + 1) * 128] = part
        colsum += np.asarray(o["colp"], dtype=np.float64).reshape(-1)

    ed = np.exp(ldiag)
    row_denom = rows - ed
    col_denom = colsum - ed
    sim12 = float(np.sum(ldiag - np.log(row_denom)))
    sim21 = float(np.sum(ldiag - np.log(col_denom)))
    result = (np.float32(-sim12), np.float32(-sim21))
    if _full_result:
        return result, bres
    return result
